# revision 66
# baseline (speedup 1.0000x reference)
"""AttentionMV pooling kernel for Trainium2 (Bass/Tile), 8-core data-parallel.

Computes, for full inputs x:(64,2048,1024) c:(64,1024) W:(1024,1) b:(2048,1)
U:(1024,2048):
    et = c @ U + (x @ W)[..., 0] + b[:, 0]        # (B, T)
    at = softmax(et, axis=-1)
    out = einsum('bt,bte->be', at, x)             # (B, E)

Sharding: data-parallel over batch B across the 8 NeuronCores (8 batches per
core); W/b/U replicated. No collectives; the host concatenates per-core
outputs.

The kernel is HBM-bound (x is 512 MiB f32), so x/c/U/W are converted to
fp16 on the host before upload — halving device HBM traffic and doubling
DVE tensor_tensor throughput. All reductions accumulate in f32 (DVE/ACT
accum_out, PSUM), keeping the end-to-end relative error ~6e-4.

Per-core dataflow (x read from HBM exactly once, in fp16):
  1. ct[t, b] = sum_e U[e,t] c[b,e] + bias[t] - SHIFT  on PE (c transposed
     on-chip via identity matmuls), stored [t%128, t//128, b] in f32.
  2. Per x t-chunk [128t x 1024e], the x@W row sum runs on a 7:9 mix of
     engines so neither exceeds the DMA floor: fused DVE
     scalar_tensor_tensor+accum_out (no DVE fast mode, ~1127ns), or DVE
     tensor_tensor at fp16 2x (~594ns) + ACT Identity+accum_out (~1040ns).
  3. exp(et + ct) on ACT -> fp16 weights ev.
  4. out[b] = sum_t ev[t] x[t,:] as accumulating PE matmuls with ev as the
     [128,1] stationary operand, re-using the resident fp16 x tiles; the
     softmax denominator comes from a ones-matmul partition reduction,
     rides in the last output column, and is divided out on the host.
"""

import os

import numpy as np

import concourse.bass as bass
import concourse.mybir as mybir
import concourse.tile as tile
from concourse import bacc
from concourse.masks import make_identity

B, T, E = 64, 2048, 1024
NCORES = 8
BL = B // NCORES  # local batches per core
P = 128
NT = T // P  # 16 t-chunks of 128
NSUB = 4  # t-chunks per DMA tile
KTILES = NT // NSUB  # x-tiles per batch
NE = E // P  # 8 e-chunks
F32 = mybir.dt.float32
F16 = mybir.dt.float16  # 10 mantissa bits: ~4x lower quantization error than bf16
SHIFT = 10.0  # softmax exp shift; cancels exactly in the normalization

_CACHE = {}
LAST_RESULTS = None  # BassKernelResults of the most recent run (for test harness)


def build_bass():
    nc = bacc.Bacc(None, target_bir_lowering=False)

    x = nc.dram_tensor("x", [BL, T, E], F16, kind="ExternalInput")
    c = nc.dram_tensor("c", [BL, E], F16, kind="ExternalInput")
    W = nc.dram_tensor("W", [E, 1], F16, kind="ExternalInput")
    bias = nc.dram_tensor("b", [T, 1], F32, kind="ExternalInput")
    U = nc.dram_tensor("U", [E, T], F16, kind="ExternalInput")
    # last column holds the softmax denominator (divided out on the host)
    out = nc.dram_tensor("out", [BL, E + 1], F32, kind="ExternalOutput")

    with tile.TileContext(nc) as tc:
        with (
            tc.tile_pool(name="big", bufs=20) as big,
            tc.tile_pool(name="singles", bufs=1) as singles,
            tc.tile_pool(name="pb", bufs=3) as pb,
            tc.tile_pool(name="psum", bufs=2, space="PSUM") as psum,
        ):
            # ---------------- constants / small inputs ----------------
            w_bc = singles.tile([P, E], F16)  # W broadcast to all partitions
            wap = W[:, 0:1]
            nc.gpsimd.dma_start(
                out=w_bc,
                in_=bass.AP(tensor=wap.tensor, offset=wap.offset, ap=[[0, P], [1, E]]),
            )

            # bias[t] laid out [t%128, t//128] so it can be an ACT per-partition bias
            bias_pt = singles.tile([P, NT], F32)
            bap = bias[:, 0:1]
            nc.gpsimd.dma_start(
                out=bias_pt,
                in_=bass.AP(tensor=bap.tensor, offset=bap.offset, ap=[[1, P], [P, NT]]),
            )

            c_sb = singles.tile([BL, E], F16)
            nc.sync.dma_start(out=c_sb, in_=c[:, :])

            id8 = singles.tile([BL, BL], F16)
            make_identity(nc, id8)

            ones_r = singles.tile([P, 2], F16)
            nc.vector.memset(ones_r, 1.0)
            # fold the fixed softmax shift into the bias that lands in ct_all
            shift_sb = singles.tile([P, 1], F32)
            nc.vector.memset(shift_sb, -SHIFT)
            nc.scalar.activation(
                out=bias_pt,
                in_=bias_pt,
                func=mybir.ActivationFunctionType.Identity,
                bias=shift_sb,
                scale=1.0,
            )

            # ---------------- transpose c: cT[e, j, b] ----------------
            cT = singles.tile([P, NE, BL], F16)
            for j in range(NE):
                tp = psum.tile([P, BL], F32, tag="tr", bufs=1)
                nc.tensor.matmul(
                    tp, lhsT=c_sb[:, j * P : (j + 1) * P], rhs=id8, start=True, stop=True
                )
                nc.scalar.copy(out=cT[:, j, :], in_=tp)

            # Interleave U with the first two batches' x tiles: x first keeps
            # the DVE fed from the start (the x@W chunks don't need ct); U
            # arrives steadily so ct is ready ~1/3 into the run, and the deep
            # et ring lets the DVE run ahead of the ct-gated exp meanwhile.
            xr = x[:, :, :].rearrange("b (k n p) e -> b k p n e", n=NSUB, p=P)
            ur = U[:, :].rearrange("(j p) t -> j p t", p=P)
            u_tiles = [None] * NE
            xts_pre = [[None] * KTILES for _ in range(BL)]
            uj = 0
            for step in range(BL * KTILES):
                bi, k = divmod(step, KTILES)
                xt = big.tile([P, NSUB, E], F16, tag="big", name=f"x{bi}_{k}")
                if step == 0:
                    # quarter-split the very first tile so the DVE starts asap
                    for q in range(NSUB):
                        nc.sync.dma_start(
                            out=xt[:, q : q + 1, :], in_=xr[bi, k, :, q : q + 1, :]
                        )
                elif step in (1, 2, 3):
                    half = NSUB // 2
                    nc.sync.dma_start(out=xt[:, :half, :], in_=xr[bi, k, :, :half, :])
                    nc.sync.dma_start(out=xt[:, half:, :], in_=xr[bi, k, :, half:, :])
                else:
                    nc.sync.dma_start(out=xt, in_=xr[bi, k])
                xts_pre[bi][k] = xt
                if step >= 3 and uj < NE:
                    ut = big.tile([P, T], F16, tag="big", name=f"u{uj}")
                    nc.sync.dma_start(out=ut, in_=ur[uj])
                    u_tiles[uj] = ut
                    uj += 1

            # ---------------- ct = U.T @ cT + bias ----------------
            # One single-bank PSUM tile holds all 16 t-chunk accumulators as
            # element-disjoint regions; U tiles are consumed as they arrive
            # (j outer), freeing their pool slots immediately.
            # start=True zeroes the whole 2KB zero-region (= this bank), so
            # only the very first matmul starts; everything else accumulates.
            ct_ps = psum.tile([P, NT, BL], F32, tag="ctacc", bufs=1)
            for j in range(NE):
                for i in range(NT):
                    nc.tensor.matmul(
                        ct_ps[:, i, :],
                        lhsT=u_tiles[j][:, i * P : (i + 1) * P],
                        rhs=cT[:, j, :],
                        start=(j == 0 and i == 0),
                        stop=(j == NE - 1 and i == NT - 1),
                    )

            # ct_all[p, i, b] = ct[i*128 + p, b] + bias[i*128 + p] - SHIFT
            ct_all = singles.tile([P, NT, BL], F32)
            for i in range(NT):
                nc.scalar.activation(
                    out=ct_all[:, i, :],
                    in_=ct_ps[:, i, :],
                    func=mybir.ActivationFunctionType.Identity,
                    bias=bias_pt[:, i : i + 1],
                    scale=1.0,
                )

            # ---------------- main loop over local batches ----------------
            # x@W row sums, mixed across engines so neither DVE nor ACT
            # exceeds the DMA floor:
            #  - stt chunks: one DVE scalar_tensor_tensor (no fast mode,
            #    ~1127ns) with fused f32 accum_out;
            #  - tt chunks: DVE tensor_tensor at fp16 2x (~594ns) into a
            #    ping-pong scratch, then an ACT Identity+accum_out row sum
            #    (~1040ns). (tensor_tensor_reduce hangs this HW's DVE ucode.)
            scratch_stt = singles.tile([P, E], F16)  # dead stt elementwise out
            scratch2 = singles.tile([P, E], F16)  # dead ACT elementwise out
            out_tiles = []
            # stt chunks per batch: ACT-heavy early (engines are DMA-starved
            # during the U phase anyway), ACT-light late so the tail chain
            # (DVE -> ACT -> store) drains fast; global balance unchanged.
            stt_sels = [
                {0, 2, 5, 8, 11, 14},             # b0: f=6
                {0, 2, 4, 7, 9, 11, 14},          # b1: f=7
                {0, 2, 4, 7, 9, 11, 14},          # b2: f=7
                {0, 2, 4, 7, 9, 11, 14},          # b3: f=7
                {0, 2, 4, 7, 9, 11, 14},          # b4: f=7
                {0, 2, 4, 7, 9, 11, 14},          # b5: f=7
                {0, 2, 4, 7, 9, 11, 14},          # b6: f=7
                {0, 2, 4, 6, 8, 10, 12, 14},      # b7: f=8
            ]

            for bi in range(BL):
                xts = xts_pre[bi]

                # Tile-granular pipeline: the softmax shift is a constant (not
                # the row max), so each t-chunk's exp contribution is
                # independent — no per-batch barrier anywhere.
                dps = psum.tile([1, 2], F32, tag="den")
                ops = psum.tile([1, E], F32, tag="out")
                for k in range(KTILES):
                    for n in range(NSUB):
                        i = k * NSUB + n
                        # et_i = sum_e x[t, e] * W[e]   (f32 accumulate)
                        et_i = pb.tile([P, 1], F32, tag="et", bufs=48, name=f"et{bi}_{i}")
                        if i in stt_sels[bi]:
                            nc.vector.scalar_tensor_tensor(
                                out=scratch_stt,
                                in0=xts[k][:, n, :],
                                scalar=0.0,
                                in1=w_bc,
                                op0=mybir.AluOpType.add,
                                op1=mybir.AluOpType.mult,
                                accum_out=et_i,
                            )
                        else:
                            prod = pb.tile(
                                [P, E], F16, tag="scr", bufs=6, name=f"pr{bi}_{i}"
                            )
                            nc.vector.tensor_tensor(
                                out=prod,
                                in0=xts[k][:, n, :],
                                in1=w_bc,
                                op=mybir.AluOpType.mult,
                            )
                            nc.scalar.activation(
                                out=scratch2,
                                in_=prod,
                                func=mybir.ActivationFunctionType.Identity,
                                accum_out=et_i,
                            )
                        # ev_i = exp(et_i + ct + bias - SHIFT)  (fp16 weights)
                        ev_i = pb.tile(
                            [P, 1], F16, tag="ev", bufs=32, name=f"ev{bi}_{i}"
                        )
                        nc.scalar.activation(
                            out=ev_i,
                            in_=et_i,
                            func=mybir.ActivationFunctionType.Exp,
                            bias=ct_all[:, i, bi : bi + 1],
                            scale=1.0,
                        )
                        # denominator contribution + weighted sum of x rows
                        nc.tensor.matmul(
                            dps,
                            lhsT=ev_i,
                            rhs=ones_r,
                            start=(i == 0),
                            stop=(i == NT - 1),
                        )
                        for h in range(2):
                            nc.tensor.matmul(
                                ops[:, h * 512 : (h + 1) * 512],
                                lhsT=ev_i,
                                rhs=xts[k][:, n, h * 512 : (h + 1) * 512],
                                start=(i == 0),
                                stop=(i == NT - 1),
                            )

                out_sb = pb.tile([1, E + 1], F32, tag="out_sb", bufs=6, name=f"osb{bi}")
                if bi == BL - 1:
                    # split the final copy across ACT and the tail-idle DVE so
                    # the program's last store fires ~0.4us sooner
                    nc.scalar.copy(out=out_sb[:, : E // 2], in_=ops[:, : E // 2])
                    nc.vector.tensor_copy(out=out_sb[:, E // 2 : E], in_=ops[:, E // 2 :])
                    nc.vector.tensor_copy(out=out_sb[:, E : E + 1], in_=dps[:, 0:1])
                else:
                    nc.scalar.copy(out=out_sb[:, :E], in_=ops)
                    nc.scalar.copy(out=out_sb[:, E : E + 1], in_=dps[:, 0:1])
                out_tiles.append(out_sb)

            # Stores at the tail of the sync queue: every x load is already
            # emitted above, so these wait on their out_sb sems without
            # blocking anything (and avoid SWDGE's Q7 descriptor writes,
            # which contend with DVE 2-port ops on the shared SBUF port).
            for bi in range(BL):
                nc.sync.dma_start(out=out[bi : bi + 1, :], in_=out_tiles[bi])

    nc.compile()
    return nc


def _get_exec():
    """Build the Bass program once and return (nc, in_names, out_names,
    out_avals, jitted _body). The multi-device shard_map path hangs through
    the axon tunnel, so we run 8 independent single-device executions
    instead (the kernel has no collectives)."""
    if "exec" in _CACHE:
        return _CACHE["exec"]

    import jax
    from concourse import bass2jax, mybir as _mybir

    bass2jax.install_neuronx_cc_hook()
    nc = build_bass()

    in_names, out_names, out_avals, zero_shapes = [], [], [], []
    for alloc in nc.m.functions[0].allocations:
        if not isinstance(alloc, _mybir.MemoryLocationSet):
            continue
        name = alloc.memorylocations[0].name
        if alloc.kind == "ExternalInput":
            in_names.append(name)
        elif alloc.kind == "ExternalOutput":
            out_names.append(name)
            shape = tuple(alloc.tensor_shape)
            dtype = _mybir.dt.np(alloc.dtype)
            out_avals.append(jax.core.ShapedArray(shape, dtype))
            zero_shapes.append((shape, dtype))
    n_params = len(in_names)
    all_names = in_names + out_names
    donate = tuple(range(n_params, n_params + len(out_names)))

    def _body(*args):
        outs = bass2jax._bass_exec_p.bind(
            *args,
            out_avals=tuple(out_avals),
            in_names=tuple(all_names),
            out_names=tuple(out_names),
            lowering_input_output_aliases=(),
            sim_require_finite=True,
            sim_require_nnan=True,
            nc=nc,
        )
        return tuple(outs)

    jitted = jax.jit(_body, donate_argnums=donate, keep_unused=True)
    _CACHE["exec"] = (nc, in_names, out_names, zero_shapes, jitted)
    return _CACHE["exec"]


_VERBOSE = os.environ.get("BASS_KERNEL_VERBOSE", "0") == "1"


def _log(msg):
    if _VERBOSE:
        import time

        print(f"[kernel {time.strftime('%H:%M:%S')}] {msg}", flush=True)


def kernel(x, c, W, b, U, trace=False, sequential=None):
    import jax

    nc, in_names, out_names, zero_shapes, jitted = _get_exec()

    f16 = np.float16
    x = np.asarray(x, dtype=np.float32).astype(f16)
    c = np.asarray(c, dtype=np.float32).astype(f16)
    W = np.asarray(W, dtype=np.float32).astype(f16)
    b = np.ascontiguousarray(b, dtype=np.float32)
    U = np.asarray(U, dtype=np.float32).astype(f16)

    if sequential is None:
        sequential = os.environ.get("BASS_KERNEL_SEQUENTIAL", "0") == "1"

    devices = jax.devices()[:NCORES]

    def _dispatch(k, dev):
        per_core = {
            "x": x[k * BL : (k + 1) * BL],
            "c": c[k * BL : (k + 1) * BL],
            "W": W,
            "b": b,
            "U": U,
        }
        if nc.partition_id_tensor is not None:
            pid = nc.partition_id_tensor
            per_core[pid.name] = np.full(pid.shape, k, dtype=mybir.dt.np(pid.dtype))
        _log(f"core {k}: device_put")
        args = [
            jax.device_put(np.ascontiguousarray(per_core[n]), dev) for n in in_names
        ]
        args += [
            jax.device_put(np.zeros(shape, dtype), dev) for shape, dtype in zero_shapes
        ]
        _log(f"core {k}: launch")
        return jitted(*args)

    def _final(res):
        # normalize on the host: out / den (den rides in the last column)
        o = res["out"].astype(np.float64)
        return (o[:, :E] / o[:, E : E + 1]).astype(np.float32)

    parts = [None] * NCORES
    if sequential:
        for k, dev in enumerate(devices):
            outs = _dispatch(k, dev)
            res = {name: np.asarray(outs[i]) for i, name in enumerate(out_names)}
            parts[k] = _final(res)
            _log(f"core {k}: done")
    else:
        futures = [_dispatch(k, dev) for k, dev in enumerate(devices)]
        for k, outs in enumerate(futures):
            res = {name: np.asarray(outs[i]) for i, name in enumerate(out_names)}
            parts[k] = _final(res)
            _log(f"core {k}: done")
    return np.concatenate(parts, axis=0)
